# revision 11
# baseline (speedup 1.0000x reference)
"""ChildSum TreeLSTM cell for 8 Trainium2 NeuronCores — self-contained kernel.

Sharding: nodes and edges are partitioned by edge_dst owner across the 8
cores (25000 nodes each). Within a core, nodes are permuted into 98 blocks
of 256 destination nodes, balanced so ~90 blocks carry at most 256 edges
(2 edge chunks) and the last 8 blocks absorb heavy nodes (384 edges, 3
chunks). During input staging the host materializes the halo — h[src],
c[src] rows for every edge in block order (plus a feature-major copy of
h[src] for the forget-gate matmul) — so the device kernel is pure
streaming DMA + fp32r matmuls. Small weights (U_f, U_iou, W_iou) are
replicated on every core. Outputs come back feature-major and permuted;
the host inverts both.

Device pipeline, per group of G=2 blocks (512 destination nodes):
  per block:  f = sigmoid(h_child @ U_f_w.T [+ U_f_b])   (per-edge, PE)
              h_tildT += h_child.T @ S ; c_aggT += (f*c_child).T @ S
                (S = one-hot dst selection built on DVE via iota/is_equal)
  per group:  iouT[fo] = sum_fi W.T[fi,fo] @ xT + U.T[fi,fo] @ h_tildT
              i,o,u = sigmoid/sigmoid/tanh(iouT + b_iou)  (ACT, bias/partition)
              c_newT = i*u + c_aggT ; h_newT = o * tanh(c_newT)
"""
import sys

for _p in ("/opt/trn_rl_repo",):
    if _p not in sys.path:
        sys.path.insert(0, _p)

import heapq

import numpy as np

import concourse.bass as bass
import concourse.bacc as bacc
import concourse.mybir as mybir
import concourse.tile as tile
from concourse.bass_utils import run_bass_kernel_spmd

f32 = mybir.dt.float32
f32r = mybir.dt.float32r
bf16 = mybir.dt.bfloat16
H_BF16 = True  # h_child paths (forget gate + h_tild segment sum) in bf16
HDT = bf16 if H_BF16 else f32r

N_CORES = 8
BLK = 256   # destination nodes per block
G = 2       # blocks per group (iou phase fusion)
H = 256
X = 256
OVER = 8    # trailing blocks with extra edge capacity

LAST_EXEC_TIME_NS = None
_PROGRAM_CACHE = {}


def _build_program(nblk, loc, ec_list, fbias_zero):
    FO = 3 * H // 128  # 6
    KH = H // 128      # 2
    GN = G * BLK       # nodes per group
    assert nblk % G == 0
    ecs = np.asarray(ec_list)
    # hslab: per block [128, ec*H (h_child, chunk-major) + KH*ec*128 (h_childT)]
    hcols = np.concatenate([[0], np.cumsum(ecs * H + KH * ecs * 128)])
    # cslab: per block [128, ec*H (c_child) + ec (dst_rel)]
    ccols = np.concatenate([[0], np.cumsum(ecs * H + ecs)])

    nc = bacc.Bacc(None, target_bir_lowering=False, debug=False)

    hsl_d = nc.declare_dram_parameter("hslab", [128, int(hcols[-1])], HDT,
                                      isOutput=False)
    csl_d = nc.declare_dram_parameter("cslab", [128, int(ccols[-1])], f32,
                                      isOutput=False)
    xT_d = nc.declare_dram_parameter("xT", [X, loc], f32r, isOutput=False)
    ufwT_d = nc.declare_dram_parameter("ufwT", [X, H], HDT, isOutput=False)
    wiouT_d = nc.declare_dram_parameter("wiouT", [X, 3 * H], f32r, isOutput=False)
    uiouT_d = nc.declare_dram_parameter("uiouT", [H, 3 * H], f32r, isOutput=False)
    bcol_d = nc.declare_dram_parameter("bcol", [128, FO], f32, isOutput=False)
    iota_d = nc.declare_dram_parameter("iota", [128, BLK], f32, isOutput=False)
    if not fbias_zero:
        ufb_d = nc.declare_dram_parameter("ufb", [1, H], HDT, isOutput=False)

    houtT_d = nc.declare_dram_parameter("houtT", [H, loc], f32, isOutput=True)
    coutT_d = nc.declare_dram_parameter("coutT", [H, loc], f32, isOutput=True)

    SIG = mybir.ActivationFunctionType.Sigmoid
    TANH = mybir.ActivationFunctionType.Tanh

    with tile.TileContext(nc) as tc:
        with (
            tc.tile_pool(name="const", bufs=1) as cpool,
            tc.tile_pool(name="io", bufs=4) as iop,
            tc.tile_pool(name="work", bufs=3) as wp,
            tc.tile_pool(name="grp", bufs=2) as gp,
            tc.tile_pool(name="ps", bufs=1, space="PSUM") as psp,
            tc.tile_pool(name="ps_acc", bufs=1, space="PSUM") as pacc,
        ):
            iota_t = cpool.tile([128, BLK], f32)
            nc.sync.dma_start(out=iota_t[:], in_=iota_d[:])
            bcol_t = cpool.tile([128, FO], f32)
            nc.sync.dma_start(out=bcol_t[:], in_=bcol_d[:])
            ufw_t = []
            for fi in range(KH):
                t = cpool.tile([128, H], HDT, tag=f"ufw{fi}", name=f"ufw{fi}")
                nc.sync.dma_start(out=t[:], in_=ufwT_d[fi * 128:(fi + 1) * 128, :])
                ufw_t.append(t)
            wiou_t = [[None] * FO for _ in range(KH)]
            uiou_t = [[None] * FO for _ in range(KH)]
            for fi in range(KH):
                for fo in range(FO):
                    t = cpool.tile([128, 128], f32r, tag=f"wiou{fi}_{fo}",
                                   name=f"wiou{fi}_{fo}")
                    nc.sync.dma_start(
                        out=t[:], in_=wiouT_d[fi * 128:(fi + 1) * 128,
                                              fo * 128:(fo + 1) * 128])
                    wiou_t[fi][fo] = t
                    t = cpool.tile([128, 128], f32r, tag=f"uiou{fi}_{fo}",
                                   name=f"uiou{fi}_{fo}")
                    nc.sync.dma_start(
                        out=t[:], in_=uiouT_d[fi * 128:(fi + 1) * 128,
                                              fo * 128:(fo + 1) * 128])
                    uiou_t[fi][fo] = t
            if not fbias_zero:
                ones_t = cpool.tile([1, 128], HDT)
                nc.vector.memset(ones_t[:].bitcast(f32 if not H_BF16 else bf16),
                                 1.0)
                ufb_t = cpool.tile([1, H], HDT)
                nc.sync.dma_start(out=ufb_t[:], in_=ufb_d[:])

            max_ec = max(ec_list)
            for g in range(nblk // G):
                xtg = gp.tile([128, KH, GN], f32r, tag="xtg")
                nc.sync.dma_start(
                    out=xtg[:],
                    in_=xT_d[:, g * GN:(g + 1) * GN].rearrange(
                        "(f p) c -> p f c", p=128))
                htg = [gp.tile([128, GN], f32r, tag=f"htg{fi}", name=f"htg{fi}")
                       for fi in range(KH)]
                cag = [gp.tile([128, GN], f32, tag=f"cag{fi}", name=f"cag{fi}")
                       for fi in range(KH)]

                for jj in range(G):
                    j = g * G + jj
                    ec = ec_list[j]
                    ne = ec * 128
                    h0 = int(hcols[j])
                    c0 = int(ccols[j])
                    hsl_t = iop.tile([128, max_ec * H * 2], HDT, tag="hsl")
                    nc.sync.dma_start(
                        out=hsl_t[:, :ec * H + KH * ne],
                        in_=hsl_d[:, h0:h0 + ec * H + KH * ne])
                    csl_t = iop.tile([128, max_ec * (H + 1)], f32, tag="csl")
                    nc.sync.dma_start(
                        out=csl_t[:, :ec * H + ec],
                        in_=csl_d[:, c0:c0 + ec * H + ec])

                    htild_ps = [pacc.tile([128, BLK], f32, tag=f"htild{fi}",
                                          name=f"htild_ps{fi}")
                                for fi in range(KH)]
                    cagg_ps = [pacc.tile([128, BLK], f32, tag=f"cagg{fi}",
                                         name=f"cagg_ps{fi}")
                               for fi in range(KH)]

                    hT0 = ec * H  # h_childT offset within hslab block
                    for ci in range(ec):
                        hch_c = hsl_t[:, ci * H:(ci + 1) * H]
                        cch_c = csl_t[:, ci * H:(ci + 1) * H]
                        s_t = wp.tile([128, BLK], f32r, tag="S")
                        nc.vector.tensor_scalar(
                            out=s_t[:], in0=iota_t[:],
                            scalar1=csl_t[:, ec * H + ci:ec * H + ci + 1],
                            scalar2=None,
                            op0=mybir.AluOpType.is_equal)
                        if H_BF16:
                            s_h = wp.tile([128, BLK], bf16, tag="Sh")
                            nc.vector.tensor_scalar(
                                out=s_h[:], in0=iota_t[:],
                                scalar1=csl_t[:, ec * H + ci:ec * H + ci + 1],
                                scalar2=None,
                                op0=mybir.AluOpType.is_equal)
                        else:
                            s_h = s_t
                        f_ps = psp.tile([128, H], f32, tag="f", bufs=2)
                        for fi in range(KH):
                            nc.tensor.matmul(
                                out=f_ps[:],
                                lhsT=hsl_t[:, hT0 + fi * ne + ci * 128:
                                           hT0 + fi * ne + ci * 128 + 128],
                                rhs=ufw_t[fi][:],
                                start=(fi == 0),
                                stop=(fi == KH - 1 and fbias_zero))
                        if not fbias_zero:
                            nc.tensor.matmul(out=f_ps[:], lhsT=ones_t[:],
                                             rhs=ufb_t[:],
                                             start=False, stop=True)
                        f_sb = wp.tile([128, H], f32, tag="fsb")
                        nc.scalar.activation(out=f_sb[:], in_=f_ps[:], func=SIG)
                        fc_t = wp.tile([128, H], f32r, tag="fc")
                        nc.vector.tensor_tensor(out=fc_t[:], in0=f_sb[:],
                                                in1=cch_c,
                                                op=mybir.AluOpType.mult)
                        for fi in range(KH):
                            nc.tensor.matmul(
                                out=htild_ps[fi][:],
                                lhsT=hch_c[:, fi * 128:(fi + 1) * 128],
                                rhs=s_h[:],
                                start=(ci == 0), stop=(ci == ec - 1))
                            nc.tensor.matmul(
                                out=cagg_ps[fi][:],
                                lhsT=fc_t[:, fi * 128:(fi + 1) * 128],
                                rhs=s_t[:],
                                start=(ci == 0), stop=(ci == ec - 1))

                    for fi in range(KH):
                        nc.vector.tensor_copy(
                            out=htg[fi][:, jj * BLK:(jj + 1) * BLK],
                            in_=htild_ps[fi][:])
                        nc.vector.tensor_copy(
                            out=cag[fi][:, jj * BLK:(jj + 1) * BLK],
                            in_=cagg_ps[fi][:])

                # ---- iou + apply for the whole group ----
                sb_act = []
                for fo in range(FO):
                    iou_ps = psp.tile([128, GN], f32, tag="iou", bufs=2)
                    first = True
                    for fi in range(KH):
                        nc.tensor.matmul(out=iou_ps[:], lhsT=wiou_t[fi][fo][:],
                                         rhs=xtg[:, fi, :], start=first,
                                         stop=False)
                        first = False
                    for fi in range(KH):
                        nc.tensor.matmul(out=iou_ps[:], lhsT=uiou_t[fi][fo][:],
                                         rhs=htg[fi][:], start=False,
                                         stop=(fi == KH - 1))
                    sb = wp.tile([128, GN], f32, tag=f"act{fo}", name=f"act{fo}")
                    nc.scalar.activation(out=sb[:], in_=iou_ps[:],
                                         func=(SIG if fo < 2 * KH else TANH),
                                         bias=bcol_t[:, fo:fo + 1])
                    sb_act.append(sb)

                for fi in range(KH):
                    i_sb, o_sb, u_sb = (sb_act[fi], sb_act[KH + fi],
                                        sb_act[2 * KH + fi])
                    iu = wp.tile([128, GN], f32, tag="iu")
                    nc.gpsimd.tensor_tensor(out=iu[:], in0=i_sb[:], in1=u_sb[:],
                                            op=mybir.AluOpType.mult)
                    cn = gp.tile([128, GN], f32, tag="cn")
                    nc.vector.tensor_tensor(out=cn[:], in0=iu[:],
                                            in1=cag[fi][:],
                                            op=mybir.AluOpType.add)
                    nc.scalar.dma_start(
                        out=coutT_d[fi * 128:(fi + 1) * 128,
                                    g * GN:(g + 1) * GN], in_=cn[:])
                    th = wp.tile([128, GN], f32, tag="th")
                    nc.scalar.activation(out=th[:], in_=cn[:], func=TANH)
                    hn = gp.tile([128, GN], f32, tag="hn")
                    nc.gpsimd.tensor_tensor(out=hn[:], in0=o_sb[:], in1=th[:],
                                            op=mybir.AluOpType.mult)
                    nc.scalar.dma_start(
                        out=houtT_d[fi * 128:(fi + 1) * 128,
                                    g * GN:(g + 1) * GN], in_=hn[:])

    nc.compile()
    return nc


def _pack_blocks(deg, nblk, caps_e):
    """Balanced bin packing: place nodes (desc degree) into blocks of 256
    node slots, respecting per-block edge capacities where possible."""
    npc = deg.shape[0]
    order = np.argsort(-deg, kind="stable")
    rem_e = caps_e.astype(np.int64).copy()
    rem_n = np.full(nblk, BLK, np.int64)
    heap = [(-rem_e[b], b) for b in range(nblk)]
    heapq.heapify(heap)
    assign = np.empty(npc, np.int64)
    for nd in order:
        d = int(deg[nd])
        tmp = []
        placed = False
        while heap:
            negre, b = heapq.heappop(heap)
            if rem_n[b] == 0:
                continue
            if d <= rem_e[b] or d == 0:
                rem_e[b] -= d
                rem_n[b] -= 1
                assign[nd] = b
                placed = True
                if rem_n[b] > 0:
                    heapq.heappush(heap, (-rem_e[b], b))
                break
            tmp.append((negre, b))
        for t in tmp:
            heapq.heappush(heap, t)
        if not placed:
            b = int(np.argmax(np.where(rem_n > 0, rem_e, -(1 << 60))))
            rem_e[b] -= d
            rem_n[b] -= 1
            assign[nd] = b
    blk_sorted = np.argsort(assign[order], kind="stable")
    perm = order[blk_sorted]  # nodes in block-major placement order
    pos = np.empty(npc, np.int64)
    used = np.bincount(assign, minlength=nblk)
    starts = np.concatenate([[0], np.cumsum(used)[:-1]])
    pos[perm] = np.arange(npc) - np.repeat(starts, used)
    return assign, pos, perm, used


def _prep_inputs(x, h, c, W_iou, U_iou, b_iou, U_f_w, U_f_b,
                 edge_src, edge_dst):
    n = x.shape[0]
    assert n % N_CORES == 0
    npc = n // N_CORES
    nblk = -(-npc // BLK)
    nblk = -(-nblk // G) * G  # group-aligned
    loc = nblk * BLK

    x = np.asarray(x, np.float32)
    h = np.asarray(h, np.float32)
    c = np.asarray(c, np.float32)
    edge_src = np.asarray(edge_src, np.int64)
    edge_dst = np.asarray(edge_dst, np.int64)

    n_over = min(OVER, nblk) if nblk > OVER else 0
    caps_e = np.array([BLK] * (nblk - n_over) + [BLK + 128] * n_over, np.int64)

    owner = edge_dst // npc
    cores = []
    ec_arr = np.zeros((N_CORES, nblk), np.int64)
    for k in range(N_CORES):
        m = owner == k
        ldst = edge_dst[m] - k * npc
        src = edge_src[m]
        deg = np.bincount(ldst, minlength=npc)
        assign, pos, perm, used = _pack_blocks(deg, nblk, caps_e)
        blk_id = assign[ldst]
        dstrel = pos[ldst]
        cnt = np.bincount(blk_id, minlength=nblk)
        ec_arr[k] = -(-np.maximum(cnt, 1) // 128)
        cores.append((src, blk_id, dstrel, cnt, perm, used))
    ec_list = tuple(int(v) for v in ec_arr.max(axis=0))
    ecs = np.asarray(ec_list)
    eoff = np.concatenate([[0], np.cumsum(ecs * 128)])
    tot_e = int(eoff[-1])
    hcols = np.concatenate([[0], np.cumsum(ecs * H + 2 * ecs * 128)])
    ccols = np.concatenate([[0], np.cumsum(ecs * H + ecs)])

    ufwT = np.ascontiguousarray(np.asarray(U_f_w, np.float32).T)
    if H_BF16:
        ufwT = ufwT.astype(mybir.dt.np(bf16))
    wiouT = np.ascontiguousarray(np.asarray(W_iou, np.float32).T)
    uiouT = np.ascontiguousarray(np.asarray(U_iou, np.float32).T)
    b_iou_f = np.asarray(b_iou, np.float32).reshape(-1)
    bcol = np.ascontiguousarray(b_iou_f.reshape(3 * H // 128, 128).T)
    iota = np.broadcast_to(np.arange(BLK, dtype=np.float32), (128, BLK)).copy()
    U_f_b_f = np.asarray(U_f_b, np.float32).reshape(-1)
    fbias_zero = not U_f_b_f.any()

    in_maps = []
    perms = []
    for k in range(N_CORES):
        src, blk_id, dstrel, cnt, perm, used = cores[k]
        start = np.zeros(nblk, np.int64)
        np.cumsum(cnt[:-1], out=start[1:])
        eorder = np.argsort(blk_id, kind="stable")
        slot_in_blk = np.arange(blk_id.size) - start[blk_id[eorder]]
        flat_pos = eoff[blk_id[eorder]] + slot_in_blk
        hch = np.zeros((tot_e, H), np.float32)
        cch = np.zeros((tot_e, H), np.float32)
        hch[flat_pos] = h[src[eorder]]
        cch[flat_pos] = c[src[eorder]]
        flat_dst = np.full(tot_e, -1.0, np.float32)
        flat_dst[flat_pos] = dstrel[eorder].astype(np.float32)

        hdt = mybir.dt.np(bf16) if H_BF16 else np.float32
        hslab = np.empty((128, int(hcols[-1])), hdt)
        cslab = np.empty((128, int(ccols[-1])), np.float32)
        for j in range(nblk):
            ec = int(ecs[j])
            ne = ec * 128
            e0, e1 = int(eoff[j]), int(eoff[j + 1])
            hb = hch[e0:e1]                       # [ne, H]
            # h_child chunk-major: [p, ci, h]
            p1 = hb.reshape(ec, 128, H).transpose(1, 0, 2).reshape(128, ec * H)
            # h_childT: [p(feat within chunk), fi, e]
            p2 = hb.T.reshape(2, 128, ne).transpose(1, 0, 2).reshape(128, 2 * ne)
            hslab[:, int(hcols[j]):int(hcols[j]) + ec * H] = p1
            hslab[:, int(hcols[j]) + ec * H:int(hcols[j + 1])] = p2
            cb = cch[e0:e1]
            q1 = cb.reshape(ec, 128, H).transpose(1, 0, 2).reshape(128, ec * H)
            cslab[:, int(ccols[j]):int(ccols[j]) + ec * H] = q1
            cslab[:, int(ccols[j]) + ec * H:int(ccols[j + 1])] = \
                flat_dst[e0:e1].reshape(ec, 128).T

        xT = np.zeros((X, loc), np.float32)
        gperm = perm + k * npc
        cols = np.concatenate([
            np.arange(s, s + u) for s, u in zip(range(0, loc, BLK), used)])
        xT[:, cols] = x[gperm].T
        im = {
            "hslab": hslab, "cslab": cslab, "xT": xT,
            "ufwT": ufwT, "wiouT": wiouT, "uiouT": uiouT,
            "bcol": bcol, "iota": iota,
        }
        if not fbias_zero:
            ufb_v = U_f_b_f.reshape(1, H)
            im["ufb"] = ufb_v.astype(mybir.dt.np(bf16)) if H_BF16 else ufb_v
        in_maps.append(im)
        perms.append((gperm, cols))

    meta = dict(n=n, npc=npc, nblk=nblk, loc=loc, ec_list=ec_list,
                fbias_zero=fbias_zero, perms=perms)
    return in_maps, meta


def kernel(x, h, c, W_iou, U_iou, b_iou, U_f_w, U_f_b, edge_src, edge_dst,
           _trace=False):
    global LAST_EXEC_TIME_NS
    in_maps, meta = _prep_inputs(x, h, c, W_iou, U_iou, b_iou, U_f_w, U_f_b,
                                 edge_src, edge_dst)
    key = (meta["nblk"], meta["loc"], meta["ec_list"], meta["fbias_zero"])
    nc = _PROGRAM_CACHE.get(key)
    if nc is None:
        nc = _build_program(meta["nblk"], meta["loc"], meta["ec_list"],
                            meta["fbias_zero"])
        _PROGRAM_CACHE[key] = nc
    res = run_bass_kernel_spmd(nc, in_maps, list(range(N_CORES)),
                               trace=_trace, trace_cores=[0] if _trace else None)
    if _trace:
        LAST_EXEC_TIME_NS = res.exec_time_ns

    n = meta["n"]
    h_new = np.empty((n, H), np.float32)
    c_new = np.empty((n, H), np.float32)
    for k in range(N_CORES):
        gperm, cols = meta["perms"][k]
        h_new[gperm] = res.results[k]["houtT"][:, cols].T
        c_new[gperm] = res.results[k]["coutT"][:, cols].T
    return h_new, c_new


# revision 12
# speedup vs baseline: 1.2897x; 1.2897x over previous
"""ChildSum TreeLSTM cell for 8 Trainium2 NeuronCores — self-contained kernel.

Sharding: nodes and edges are partitioned by edge_dst owner across the 8
cores (25000 nodes each). Within a core, nodes are permuted into 98 blocks
of 256 destination nodes, balanced so ~90 blocks carry at most 256 edges
(2 edge chunks) and the last 8 blocks absorb heavy nodes (384 edges, 3
chunks). During input staging the host materializes the halo — h[src],
c[src] rows for every edge in block order (plus a feature-major copy of
h[src] for the forget-gate matmul) — so the device kernel is pure
streaming DMA + fp32r matmuls. Small weights (U_f, U_iou, W_iou) are
replicated on every core. Outputs come back feature-major and permuted;
the host inverts both.

Device pipeline, per group of G=2 blocks (512 destination nodes):
  per block:  f = sigmoid(h_child @ U_f_w.T [+ U_f_b])   (per-edge, PE)
              h_tildT += h_child.T @ S ; c_aggT += (f*c_child).T @ S
                (S = one-hot dst selection built on DVE via iota/is_equal)
  per group:  iouT[fo] = sum_fi W.T[fi,fo] @ xT + U.T[fi,fo] @ h_tildT
              i,o,u = sigmoid/sigmoid/tanh(iouT + b_iou)  (ACT, bias/partition)
              c_newT = i*u + c_aggT ; h_newT = o * tanh(c_newT)
"""
import sys

for _p in ("/opt/trn_rl_repo",):
    if _p not in sys.path:
        sys.path.insert(0, _p)

import heapq

import numpy as np

import concourse.bass as bass
import concourse.bacc as bacc
import concourse.mybir as mybir
import concourse.tile as tile
from concourse.bass_utils import run_bass_kernel_spmd

f32 = mybir.dt.float32
f32r = mybir.dt.float32r

N_CORES = 8
BLK = 256   # destination nodes per block
G = 2       # blocks per group (iou phase fusion)
H = 256
X = 256
OVER = 8    # trailing blocks with extra edge capacity

LAST_EXEC_TIME_NS = None
_PROGRAM_CACHE = {}


def _build_program(nblk, loc, ec_list, fbias_zero):
    FO = 3 * H // 128  # 6
    KH = H // 128      # 2
    GN = G * BLK       # nodes per group
    assert nblk % G == 0
    ecs = np.asarray(ec_list)
    # hslab: per block [128, ec*H (h_child, chunk-major) + KH*ec*128 (h_childT)]
    hcols = np.concatenate([[0], np.cumsum(ecs * H + KH * ecs * 128)])
    # cslab: per block [128, ec*H (c_child) + ec (dst_rel)]
    ccols = np.concatenate([[0], np.cumsum(ecs * H + ecs)])

    nc = bacc.Bacc(None, target_bir_lowering=False, debug=False)

    hsl_d = nc.declare_dram_parameter("hslab", [128, int(hcols[-1])], f32r,
                                      isOutput=False)
    csl_d = nc.declare_dram_parameter("cslab", [128, int(ccols[-1])], f32,
                                      isOutput=False)
    xT_d = nc.declare_dram_parameter("xT", [X, loc], f32r, isOutput=False)
    ufwT_d = nc.declare_dram_parameter("ufwT", [X, H], f32r, isOutput=False)
    wiouT_d = nc.declare_dram_parameter("wiouT", [X, 3 * H], f32r, isOutput=False)
    uiouT_d = nc.declare_dram_parameter("uiouT", [H, 3 * H], f32r, isOutput=False)
    bcol_d = nc.declare_dram_parameter("bcol", [128, FO], f32, isOutput=False)
    iota_d = nc.declare_dram_parameter("iota", [128, BLK], f32, isOutput=False)
    if not fbias_zero:
        ufb_d = nc.declare_dram_parameter("ufb", [1, H], f32r, isOutput=False)

    houtT_d = nc.declare_dram_parameter("houtT", [H, loc], f32, isOutput=True)
    coutT_d = nc.declare_dram_parameter("coutT", [H, loc], f32, isOutput=True)

    SIG = mybir.ActivationFunctionType.Sigmoid
    TANH = mybir.ActivationFunctionType.Tanh

    with tile.TileContext(nc) as tc:
        with (
            tc.tile_pool(name="const", bufs=1) as cpool,
            tc.tile_pool(name="io", bufs=5) as iop,
            tc.tile_pool(name="work", bufs=3) as wp,
            tc.tile_pool(name="grp", bufs=2) as gp,
            tc.tile_pool(name="ps", bufs=1, space="PSUM") as psp,
            tc.tile_pool(name="ps_acc", bufs=1, space="PSUM") as pacc,
        ):
            iota_t = cpool.tile([128, BLK], f32)
            nc.sync.dma_start(out=iota_t[:], in_=iota_d[:])
            bcol_t = cpool.tile([128, FO], f32)
            nc.sync.dma_start(out=bcol_t[:], in_=bcol_d[:])
            ufw_t = []
            for fi in range(KH):
                t = cpool.tile([128, H], f32r, tag=f"ufw{fi}", name=f"ufw{fi}")
                nc.sync.dma_start(out=t[:], in_=ufwT_d[fi * 128:(fi + 1) * 128, :])
                ufw_t.append(t)
            wiou_t = [[None] * FO for _ in range(KH)]
            uiou_t = [[None] * FO for _ in range(KH)]
            for fi in range(KH):
                for fo in range(FO):
                    t = cpool.tile([128, 128], f32r, tag=f"wiou{fi}_{fo}",
                                   name=f"wiou{fi}_{fo}")
                    nc.sync.dma_start(
                        out=t[:], in_=wiouT_d[fi * 128:(fi + 1) * 128,
                                              fo * 128:(fo + 1) * 128])
                    wiou_t[fi][fo] = t
                    t = cpool.tile([128, 128], f32r, tag=f"uiou{fi}_{fo}",
                                   name=f"uiou{fi}_{fo}")
                    nc.sync.dma_start(
                        out=t[:], in_=uiouT_d[fi * 128:(fi + 1) * 128,
                                              fo * 128:(fo + 1) * 128])
                    uiou_t[fi][fo] = t
            if not fbias_zero:
                ones_t = cpool.tile([1, 128], f32r)
                nc.vector.memset(ones_t[:].bitcast(f32), 1.0)
                ufb_t = cpool.tile([1, H], f32r)
                nc.sync.dma_start(out=ufb_t[:], in_=ufb_d[:])

            max_ec = max(ec_list)
            for g in range(nblk // G):
                xtg = gp.tile([128, KH, GN], f32r, tag="xtg", bufs=3)
                nc.sync.dma_start(
                    out=xtg[:],
                    in_=xT_d[:, g * GN:(g + 1) * GN].rearrange(
                        "(f p) c -> p f c", p=128))
                htg = [gp.tile([128, GN], f32r, tag=f"htg{fi}", name=f"htg{fi}")
                       for fi in range(KH)]
                cag = [gp.tile([128, GN], f32, tag=f"cag{fi}", name=f"cag{fi}")
                       for fi in range(KH)]

                for jj in range(G):
                    j = g * G + jj
                    ec = ec_list[j]
                    ne = ec * 128
                    h0 = int(hcols[j])
                    c0 = int(ccols[j])
                    hsl_t = iop.tile([128, max_ec * H * 2], f32r, tag="hsl")
                    nc.sync.dma_start(
                        out=hsl_t[:, :ec * H + KH * ne],
                        in_=hsl_d[:, h0:h0 + ec * H + KH * ne])
                    csl_t = iop.tile([128, max_ec * (H + 1)], f32, tag="csl")
                    nc.sync.dma_start(
                        out=csl_t[:, :ec * H + ec],
                        in_=csl_d[:, c0:c0 + ec * H + ec])

                    htild_ps = [pacc.tile([128, BLK], f32, tag=f"htild{fi}",
                                          name=f"htild_ps{fi}")
                                for fi in range(KH)]
                    cagg_ps = [pacc.tile([128, BLK], f32, tag=f"cagg{fi}",
                                         name=f"cagg_ps{fi}")
                               for fi in range(KH)]

                    hT0 = ec * H  # h_childT offset within hslab block
                    for ci in range(ec):
                        hch_c = hsl_t[:, ci * H:(ci + 1) * H]
                        cch_c = csl_t[:, ci * H:(ci + 1) * H]
                        s_t = wp.tile([128, BLK], f32r, tag="S")
                        nc.vector.tensor_scalar(
                            out=s_t[:], in0=iota_t[:],
                            scalar1=csl_t[:, ec * H + ci:ec * H + ci + 1],
                            scalar2=None,
                            op0=mybir.AluOpType.is_equal)
                        f_ps = psp.tile([128, H], f32, tag="f", bufs=2)
                        for fi in range(KH):
                            nc.tensor.matmul(
                                out=f_ps[:],
                                lhsT=hsl_t[:, hT0 + fi * ne + ci * 128:
                                           hT0 + fi * ne + ci * 128 + 128],
                                rhs=ufw_t[fi][:],
                                start=(fi == 0),
                                stop=(fi == KH - 1 and fbias_zero))
                        if not fbias_zero:
                            nc.tensor.matmul(out=f_ps[:], lhsT=ones_t[:],
                                             rhs=ufb_t[:],
                                             start=False, stop=True)
                        f_sb = wp.tile([128, H], f32, tag="fsb")
                        nc.scalar.activation(out=f_sb[:], in_=f_ps[:], func=SIG)
                        fc_t = wp.tile([128, H], f32r, tag="fc")
                        nc.gpsimd.tensor_tensor(out=fc_t[:], in0=f_sb[:],
                                                in1=cch_c,
                                                op=mybir.AluOpType.mult)
                        for fi in range(KH):
                            nc.tensor.matmul(
                                out=htild_ps[fi][:],
                                lhsT=hch_c[:, fi * 128:(fi + 1) * 128],
                                rhs=s_t[:],
                                start=(ci == 0), stop=(ci == ec - 1))
                            nc.tensor.matmul(
                                out=cagg_ps[fi][:],
                                lhsT=fc_t[:, fi * 128:(fi + 1) * 128],
                                rhs=s_t[:],
                                start=(ci == 0), stop=(ci == ec - 1))

                    for fi in range(KH):
                        nc.vector.tensor_copy(
                            out=htg[fi][:, jj * BLK:(jj + 1) * BLK],
                            in_=htild_ps[fi][:])
                        nc.vector.tensor_copy(
                            out=cag[fi][:, jj * BLK:(jj + 1) * BLK],
                            in_=cagg_ps[fi][:])

                # ---- iou + apply for the whole group ----
                sb_act = []
                for fo in range(FO):
                    iou_ps = psp.tile([128, GN], f32, tag="iou", bufs=2)
                    first = True
                    for fi in range(KH):
                        nc.tensor.matmul(out=iou_ps[:], lhsT=wiou_t[fi][fo][:],
                                         rhs=xtg[:, fi, :], start=first,
                                         stop=False)
                        first = False
                    for fi in range(KH):
                        nc.tensor.matmul(out=iou_ps[:], lhsT=uiou_t[fi][fo][:],
                                         rhs=htg[fi][:], start=False,
                                         stop=(fi == KH - 1))
                    sb = wp.tile([128, GN], f32, tag=f"act{fo}", name=f"act{fo}")
                    nc.scalar.activation(out=sb[:], in_=iou_ps[:],
                                         func=(SIG if fo < 2 * KH else TANH),
                                         bias=bcol_t[:, fo:fo + 1])
                    sb_act.append(sb)

                for fi in range(KH):
                    i_sb, o_sb, u_sb = (sb_act[fi], sb_act[KH + fi],
                                        sb_act[2 * KH + fi])
                    iu = wp.tile([128, GN], f32, tag="iu")
                    nc.vector.tensor_tensor(out=iu[:], in0=i_sb[:], in1=u_sb[:],
                                            op=mybir.AluOpType.mult)
                    cn = gp.tile([128, GN], f32, tag="cn")
                    nc.vector.tensor_tensor(out=cn[:], in0=iu[:],
                                            in1=cag[fi][:],
                                            op=mybir.AluOpType.add)
                    nc.scalar.dma_start(
                        out=coutT_d[fi * 128:(fi + 1) * 128,
                                    g * GN:(g + 1) * GN], in_=cn[:])
                    th = wp.tile([128, GN], f32, tag="th")
                    nc.scalar.activation(out=th[:], in_=cn[:], func=TANH)
                    hn = gp.tile([128, GN], f32, tag="hn")
                    nc.gpsimd.tensor_tensor(out=hn[:], in0=o_sb[:], in1=th[:],
                                            op=mybir.AluOpType.mult)
                    nc.scalar.dma_start(
                        out=houtT_d[fi * 128:(fi + 1) * 128,
                                    g * GN:(g + 1) * GN], in_=hn[:])

    nc.compile()
    return nc


def _pack_blocks(deg, nblk, caps_e):
    """Balanced bin packing: place nodes (desc degree) into blocks of 256
    node slots, respecting per-block edge capacities where possible."""
    npc = deg.shape[0]
    order = np.argsort(-deg, kind="stable")
    rem_e = caps_e.astype(np.int64).copy()
    rem_n = np.full(nblk, BLK, np.int64)
    heap = [(-rem_e[b], b) for b in range(nblk)]
    heapq.heapify(heap)
    assign = np.empty(npc, np.int64)
    for nd in order:
        d = int(deg[nd])
        tmp = []
        placed = False
        while heap:
            negre, b = heapq.heappop(heap)
            if rem_n[b] == 0:
                continue
            if d <= rem_e[b] or d == 0:
                rem_e[b] -= d
                rem_n[b] -= 1
                assign[nd] = b
                placed = True
                if rem_n[b] > 0:
                    heapq.heappush(heap, (-rem_e[b], b))
                break
            tmp.append((negre, b))
        for t in tmp:
            heapq.heappush(heap, t)
        if not placed:
            b = int(np.argmax(np.where(rem_n > 0, rem_e, -(1 << 60))))
            rem_e[b] -= d
            rem_n[b] -= 1
            assign[nd] = b
    blk_sorted = np.argsort(assign[order], kind="stable")
    perm = order[blk_sorted]  # nodes in block-major placement order
    pos = np.empty(npc, np.int64)
    used = np.bincount(assign, minlength=nblk)
    starts = np.concatenate([[0], np.cumsum(used)[:-1]])
    pos[perm] = np.arange(npc) - np.repeat(starts, used)
    return assign, pos, perm, used


def _prep_inputs(x, h, c, W_iou, U_iou, b_iou, U_f_w, U_f_b,
                 edge_src, edge_dst):
    n = x.shape[0]
    assert n % N_CORES == 0
    npc = n // N_CORES
    nblk = -(-npc // BLK)
    nblk = -(-nblk // G) * G  # group-aligned
    loc = nblk * BLK

    x = np.asarray(x, np.float32)
    h = np.asarray(h, np.float32)
    c = np.asarray(c, np.float32)
    edge_src = np.asarray(edge_src, np.int64)
    edge_dst = np.asarray(edge_dst, np.int64)

    n_over = min(OVER, nblk) if nblk > OVER else 0
    caps_e = np.array([BLK] * (nblk - n_over) + [BLK + 128] * n_over, np.int64)

    owner = edge_dst // npc
    cores = []
    ec_arr = np.zeros((N_CORES, nblk), np.int64)
    for k in range(N_CORES):
        m = owner == k
        ldst = edge_dst[m] - k * npc
        src = edge_src[m]
        deg = np.bincount(ldst, minlength=npc)
        assign, pos, perm, used = _pack_blocks(deg, nblk, caps_e)
        blk_id = assign[ldst]
        dstrel = pos[ldst]
        cnt = np.bincount(blk_id, minlength=nblk)
        ec_arr[k] = -(-np.maximum(cnt, 1) // 128)
        cores.append((src, blk_id, dstrel, cnt, perm, used))
    ec_list = tuple(int(v) for v in ec_arr.max(axis=0))
    ecs = np.asarray(ec_list)
    eoff = np.concatenate([[0], np.cumsum(ecs * 128)])
    tot_e = int(eoff[-1])
    hcols = np.concatenate([[0], np.cumsum(ecs * H + 2 * ecs * 128)])
    ccols = np.concatenate([[0], np.cumsum(ecs * H + ecs)])

    ufwT = np.ascontiguousarray(np.asarray(U_f_w, np.float32).T)
    wiouT = np.ascontiguousarray(np.asarray(W_iou, np.float32).T)
    uiouT = np.ascontiguousarray(np.asarray(U_iou, np.float32).T)
    b_iou_f = np.asarray(b_iou, np.float32).reshape(-1)
    bcol = np.ascontiguousarray(b_iou_f.reshape(3 * H // 128, 128).T)
    iota = np.broadcast_to(np.arange(BLK, dtype=np.float32), (128, BLK)).copy()
    U_f_b_f = np.asarray(U_f_b, np.float32).reshape(-1)
    fbias_zero = not U_f_b_f.any()

    in_maps = []
    perms = []
    for k in range(N_CORES):
        src, blk_id, dstrel, cnt, perm, used = cores[k]
        start = np.zeros(nblk, np.int64)
        np.cumsum(cnt[:-1], out=start[1:])
        eorder = np.argsort(blk_id, kind="stable")
        slot_in_blk = np.arange(blk_id.size) - start[blk_id[eorder]]
        flat_pos = eoff[blk_id[eorder]] + slot_in_blk
        hch = np.zeros((tot_e, H), np.float32)
        cch = np.zeros((tot_e, H), np.float32)
        hch[flat_pos] = h[src[eorder]]
        cch[flat_pos] = c[src[eorder]]
        flat_dst = np.full(tot_e, -1.0, np.float32)
        flat_dst[flat_pos] = dstrel[eorder].astype(np.float32)

        hslab = np.empty((128, int(hcols[-1])), np.float32)
        cslab = np.empty((128, int(ccols[-1])), np.float32)
        for j in range(nblk):
            ec = int(ecs[j])
            ne = ec * 128
            e0, e1 = int(eoff[j]), int(eoff[j + 1])
            hb = hch[e0:e1]                       # [ne, H]
            # h_child chunk-major: [p, ci, h]
            p1 = hb.reshape(ec, 128, H).transpose(1, 0, 2).reshape(128, ec * H)
            # h_childT: [p(feat within chunk), fi, e]
            p2 = hb.T.reshape(2, 128, ne).transpose(1, 0, 2).reshape(128, 2 * ne)
            hslab[:, int(hcols[j]):int(hcols[j]) + ec * H] = p1
            hslab[:, int(hcols[j]) + ec * H:int(hcols[j + 1])] = p2
            cb = cch[e0:e1]
            q1 = cb.reshape(ec, 128, H).transpose(1, 0, 2).reshape(128, ec * H)
            cslab[:, int(ccols[j]):int(ccols[j]) + ec * H] = q1
            cslab[:, int(ccols[j]) + ec * H:int(ccols[j + 1])] = \
                flat_dst[e0:e1].reshape(ec, 128).T

        xT = np.zeros((X, loc), np.float32)
        gperm = perm + k * npc
        cols = np.concatenate([
            np.arange(s, s + u) for s, u in zip(range(0, loc, BLK), used)])
        xT[:, cols] = x[gperm].T
        im = {
            "hslab": hslab, "cslab": cslab, "xT": xT,
            "ufwT": ufwT, "wiouT": wiouT, "uiouT": uiouT,
            "bcol": bcol, "iota": iota,
        }
        if not fbias_zero:
            im["ufb"] = U_f_b_f.reshape(1, H)
        in_maps.append(im)
        perms.append((gperm, cols))

    meta = dict(n=n, npc=npc, nblk=nblk, loc=loc, ec_list=ec_list,
                fbias_zero=fbias_zero, perms=perms)
    return in_maps, meta


def kernel(x, h, c, W_iou, U_iou, b_iou, U_f_w, U_f_b, edge_src, edge_dst,
           _trace=False):
    global LAST_EXEC_TIME_NS
    in_maps, meta = _prep_inputs(x, h, c, W_iou, U_iou, b_iou, U_f_w, U_f_b,
                                 edge_src, edge_dst)
    key = (meta["nblk"], meta["loc"], meta["ec_list"], meta["fbias_zero"])
    nc = _PROGRAM_CACHE.get(key)
    if nc is None:
        nc = _build_program(meta["nblk"], meta["loc"], meta["ec_list"],
                            meta["fbias_zero"])
        _PROGRAM_CACHE[key] = nc
    res = run_bass_kernel_spmd(nc, in_maps, list(range(N_CORES)),
                               trace=_trace, trace_cores=[0] if _trace else None)
    if _trace:
        LAST_EXEC_TIME_NS = res.exec_time_ns

    n = meta["n"]
    h_new = np.empty((n, H), np.float32)
    c_new = np.empty((n, H), np.float32)
    for k in range(N_CORES):
        gperm, cols = meta["perms"][k]
        h_new[gperm] = res.results[k]["houtT"][:, cols].T
        c_new[gperm] = res.results[k]["coutT"][:, cols].T
    return h_new, c_new
